# revision 3
# baseline (speedup 1.0000x reference)
"""Trainium2 Bass kernel for nn_DiagKernel: out = x * diag(kernel).

Data-parallel over 8 NeuronCores: x [8192, 4096] f32 is sharded along the
batch dim (1024 rows per core); only the N-length diagonal of the kernel
matrix is live, so it is extracted host-side and replicated to every core.
Each core broadcast-loads the diagonal across 128 SBUF partitions once, then
streams row-tiles of x through SBUF with an elementwise DVE multiply.
"""

import numpy as np

import concourse.bacc as bacc
import concourse.mybir as mybir
from concourse import tile
from concourse.bass_utils import run_bass_kernel_spmd

N = 4096          # feature dim (columns of x; length of live diagonal)
B = 8192          # full batch
N_CORES = 8
ROWS = B // N_CORES   # rows per core
P = 128               # SBUF partitions
TILE_ROWS = P
N_TILES = ROWS // TILE_ROWS  # 8 tiles of [128, 4096] (2 MiB) per core

_nc_cache = None


def _build():
    nc = bacc.Bacc(
        "TRN2",
        target_bir_lowering=False,
        debug=False,
        num_devices=N_CORES,
    )
    x = nc.dram_tensor("x", [ROWS, N], mybir.dt.float32, kind="ExternalInput").ap()
    d = nc.dram_tensor("d", [1, N], mybir.dt.float32, kind="ExternalInput").ap()
    y = nc.dram_tensor("y", [ROWS, N], mybir.dt.float32, kind="ExternalOutput").ap()

    BANK = 512  # f32 elements per PSUM bank per partition
    with tile.TileContext(nc) as tc:
        with (
            tc.tile_pool(name="const", bufs=1) as cpool,
            tc.tile_pool(name="psum", bufs=1, space="PSUM") as ppool,
            tc.tile_pool(name="io", bufs=6) as pool,
        ):
            # Broadcast the diagonal across all 128 partitions without
            # spending DMA bandwidth on it: load the [1, N] row once
            # (16 KiB), then ones[1,128].T @ d[1,N] on the PE replicates it
            # into PSUM. The muls read d directly from PSUM (DVE may read
            # one PSUM operand).
            d_row = cpool.tile([1, N], mybir.dt.float32)
            # On the ACT ring: keeps the SP ring free so the first big x
            # load issues immediately.
            nc.scalar.dma_start(out=d_row[:], in_=d[:])
            ones = cpool.tile([1, P], mybir.dt.float32)
            nc.vector.memset(ones[:], 1.0)
            d_ps = ppool.tile([P, N], mybir.dt.float32)
            for j in range(N // BANK):
                nc.tensor.matmul(
                    d_ps[:, j * BANK : (j + 1) * BANK],
                    ones[:],
                    d_row[:, j * BANK : (j + 1) * BANK],
                )
            for i in range(N_TILES):
                t = pool.tile([P, N], mybir.dt.float32)
                # Loads on the SP HWDGE ring, stores on the ACT ring so the
                # two streams don't serialize behind each other.
                nc.sync.dma_start(out=t[:], in_=x[i * P : (i + 1) * P, :])
                nc.vector.tensor_mul(out=t[:], in0=t[:], in1=d_ps[:])
                nc.scalar.dma_start(out=y[i * P : (i + 1) * P, :], in_=t[:])

    nc.compile()
    return nc


def _get_nc():
    global _nc_cache
    if _nc_cache is None:
        _nc_cache = _build()
    return _nc_cache


def _run(x, kernel, trace=False):
    x = np.ascontiguousarray(np.asarray(x, dtype=np.float32))
    k = np.asarray(kernel, dtype=np.float32)
    assert x.shape == (B, N), x.shape
    assert k.shape == (N, N), k.shape
    d = np.ascontiguousarray(np.diagonal(k)).reshape(1, N)

    nc = _get_nc()
    in_maps = [
        {"x": x[c * ROWS : (c + 1) * ROWS], "d": d} for c in range(N_CORES)
    ]
    res = run_bass_kernel_spmd(
        nc, in_maps, core_ids=list(range(N_CORES)), trace=trace
    )
    out = np.concatenate([r["y"] for r in res.results], axis=0)
    return out, res


def kernel(x, kernel):
    out, _ = _run(x, kernel, trace=False)
    return out


def run_traced(x, kernel):
    """Test harness entry: returns (out, BassKernelResults with exec_time_ns)."""
    return _run(x, kernel, trace=True)
